# revision 23
# baseline (speedup 1.0000x reference)
import sys, os
sys.path.insert(0, "/opt/trn_rl_repo")
import numpy as np

M, L, B, C, H = 7, 12, 8, 768, 12
HD = C // H
R = 8
P, IMG = 16, 224
GRID = IMG // P
N0 = GRID * GRID + 1
LORA, ADH, NCL = 8, 64, 100
SCALE = HD ** -0.5
EPS = 1e-6
NEG = -1.0e30


# ----------------------------------------------------------------------------
# Host-side weight preprocessing: fold LN affines / lora consts / biases into
# GEMM weights, pre-block everything into the SBUF tile layouts the device
# program DMAs directly.
# ----------------------------------------------------------------------------
def _blk_lhsT(w):
    # w [K, Mo] -> [K//128, Mo//128, 128, 128] lhsT tiles (kb, mb, p, f)
    K, Mo = w.shape
    return np.ascontiguousarray(
        w.reshape(K // 128, 128, Mo // 128, 128).transpose(0, 2, 1, 3))


def _prep_tables(inp):
    f32 = np.float32
    t = {}
    n1w = inp["norm1_w"].astype(f32)
    n1b = inp["norm1_b"].astype(f32)
    qkv_w = inp["qkv_w"].astype(f32)
    qkv_b = inp["qkv_b"].astype(f32)
    la = inp["lora_a"].astype(f32)
    lb = inp["lora_b"].astype(f32)
    pw = inp["proj_w"].astype(f32)
    pb = inp["proj_b"].astype(f32)

    WQK = np.zeros((L, M, 6, 12, 128, 128), f32)
    WV = np.zeros((L, M, 6, 2, 128, 384), f32)
    LA = np.zeros((L, M, 128, 6, LORA), f32)
    LBQK = np.zeros((L, M, LORA, 1536), f32)
    LBV = np.zeros((L, M, LORA, 768), f32)
    BQK = np.zeros((L, M, 128, 12), f32)
    BV = np.zeros((L, M, 1, 768), f32)
    WP = np.zeros((L, M, 6, 6, 128, 128), f32)
    BP = np.zeros((L, M, 128, 6), f32)
    for i in range(L):
        for m in range(M):
            weff = n1w[m, i][:, None] * qkv_w[m, i]
            laeff = n1w[m, i][:, None] * la[m, i]
            beff = (qkv_b[m, i] + n1b[m, i] @ qkv_w[m, i]
                    + (n1b[m, i] @ la[m, i]) @ lb[m, i])
            WQK[i, m] = _blk_lhsT(weff[:, :1536])
            wv = weff[:, 1536:]
            WV[i, m] = wv.reshape(6, 128, 2, 384).transpose(0, 2, 1, 3)
            LA[i, m] = laeff.reshape(6, 128, LORA).transpose(1, 0, 2)
            LBQK[i, m] = lb[m, i][:, :1536]
            LBV[i, m] = lb[m, i][:, 1536:]
            BQK[i, m] = beff[:1536].reshape(12, 128).T
            BV[i, m, 0] = beff[1536:]
            WP[i, m] = _blk_lhsT(pw[m, i])
            BP[i, m] = pb[m, i].reshape(6, 128).T
    t["WQK"], t["WV"], t["LA"], t["LBQK"], t["LBV"] = WQK, WV, LA, LBQK, LBV
    t["BQK"], t["BV"], t["WP"], t["BP"] = BQK, BV, WP, BP

    t["RT"] = np.ascontiguousarray(
        inp["routers"].astype(f32).reshape(L, 6, 128, M).transpose(0, 2, 1, 3))

    n2w = inp["norm2_w"].astype(f32)
    n2b = inp["norm2_b"].astype(f32)
    fc1w = inp["fc1_w"].astype(f32)
    fc1b = inp["fc1_b"].astype(f32)
    fc2w = inp["fc2_w"].astype(f32)
    fc2b = inp["fc2_b"].astype(f32)
    adw = inp["ad_dw"].astype(f32)
    adb = inp["ad_db"].astype(f32)
    adu = inp["ad_uw"].astype(f32)
    aub = inp["ad_ub"].astype(f32)
    FC1 = np.zeros((L, 6, 24, 128, 128), f32)
    FB1 = np.zeros((L, 128, 24), f32)
    FC2 = np.zeros((L, 24, 6, 128, 128), f32)
    ADW = np.zeros((L, 128, 6, ADH), f32)
    ADB = np.zeros((L, ADH, 1), f32)
    ADU = np.zeros((L, ADH, 768), f32)
    BC = np.zeros((L, 128, 6), f32)
    for i in range(L):
        w1eff = n2w[i][:, None] * fc1w[i]
        b1eff = fc1b[i] + n2b[i] @ fc1w[i]
        FC1[i] = _blk_lhsT(w1eff)
        FB1[i] = b1eff.reshape(24, 128).T
        FC2[i] = _blk_lhsT(fc2w[i])
        ADW[i] = adw[i].reshape(6, 128, ADH).transpose(1, 0, 2)
        ADB[i] = (adb[i] + fc2b[i] @ adw[i]).reshape(ADH, 1)
        ADU[i] = adu[i]
        BC[i] = (fc2b[i] + aub[i]).reshape(6, 128).T
    t["FC1"], t["FB1"], t["FC2"] = FC1, FB1, FC2
    t["ADW"], t["ADB"], t["ADU"], t["BC"] = ADW, ADB, ADU, BC

    t["PW"] = _blk_lhsT(inp["patch_w"].astype(f32))
    pos = inp["pos_embed"].astype(f32)[0].copy()
    pos[0] += inp["cls_token"].astype(f32)[0, 0]
    pos[1:] += inp["patch_b"].astype(f32)[None, :]
    t["POS"] = np.ascontiguousarray(
        pos.T.reshape(6, 128, 197).transpose(1, 0, 2))

    nfw = inp["normf_w"].astype(f32)
    nfb = inp["normf_b"].astype(f32)
    hw = inp["head_w"].astype(f32)
    hb = inp["head_b"].astype(f32)
    hw_all = hw.transpose(1, 0, 2).reshape(C, M * NCL)
    hweff = nfw[:, None] * hw_all
    hbeff = hb.reshape(1, M * NCL) + (nfb @ hw_all)[None]
    t["HW"] = np.ascontiguousarray(
        hweff.reshape(6, 128, 700).transpose(1, 0, 2))
    t["HB"] = hbeff.astype(f32)
    return t


_TBL_SHAPES = {
    "WQK": (L, M, 6, 12, 128, 128), "WV": (L, M, 6, 2, 128, 384),
    "LA": (L, M, 128, 6, LORA), "LBQK": (L, M, LORA, 1536),
    "LBV": (L, M, LORA, 768), "BQK": (L, M, 128, 12), "BV": (L, M, 1, 768),
    "WP": (L, M, 6, 6, 128, 128), "BP": (L, M, 128, 6),
    "RT": (L, 128, 6, M),
    "FC1": (L, 6, 24, 128, 128), "FB1": (L, 128, 24),
    "FC2": (L, 24, 6, 128, 128),
    "ADW": (L, 128, 6, ADH), "ADB": (L, ADH, 1), "ADU": (L, ADH, 768),
    "BC": (L, 128, 6),
    "PW": (6, 6, 128, 128), "POS": (128, 6, 197),
    "HW": (128, 6, 700), "HB": (1, 700),
}


# ----------------------------------------------------------------------------
# Device program (per core: one sample, all 7 models, 12 layers, head).
# ----------------------------------------------------------------------------
def build_nc(n_layers=L, dbg=False):
    import concourse.bacc as bacc
    import concourse.mybir as mybir
    import concourse.tile as tile
    from concourse.masks import make_identity

    dt = mybir.dt
    AF = mybir.ActivationFunctionType
    OP = mybir.AluOpType

    nc = bacc.Bacc("TRN2", target_bir_lowering=False, debug=False,
                   num_devices=8)
    dr = {k: nc.dram_tensor(k, list(s), dt.float32, kind="ExternalInput")
          for k, s in _TBL_SHAPES.items()}
    pch = nc.dram_tensor("PCH", [128, 6, 196], dt.float32,
                         kind="ExternalInput")
    out = nc.dram_tensor("OUT", [1, 700], dt.float32, kind="ExternalOutput")
    if dbg:
        dxc = nc.dram_tensor("DXC", [128, 6, 197], dt.float32,
                             kind="ExternalOutput")
        dsz = nc.dram_tensor("DSZ", [M, 1, 197], dt.float32,
                             kind="ExternalOutput")
        dix = nc.dram_tensor("DIX", [M, 1, 16], dt.float32,
                             kind="ExternalOutput")
        dd2 = nc.dram_tensor("DD2", [M, 8, 1], dt.float32,
                             kind="ExternalOutput")
        dnm = nc.dram_tensor("DNM", [M, 1, 100], dt.float32,
                             kind="ExternalOutput")
        dmet = nc.dram_tensor("DMET", [M, 64, 197], dt.float32,
                             kind="ExternalOutput")
        dxn = nc.dram_tensor("DXN", [128, 6, 197], dt.float32,
                             kind="ExternalOutput")
        dqk2 = nc.dram_tensor("DQK2", [M, 128, 12, 197], dt.float32,
                              kind="ExternalOutput")
        dlo = nc.dram_tensor("DLO", [M, LORA, 197], dt.float32,
                             kind="ExternalOutput")
        dqk = nc.dram_tensor("DQK", [M, 128, 197], dt.float32,
                             kind="ExternalOutput")

    with tile.TileContext(nc) as tc:
        with (
            tc.tile_pool(name="const", bufs=1) as cp,
            tc.tile_pool(name="state", bufs=1) as sp,
            tc.tile_pool(name="wst", bufs=4) as wp,
            tc.tile_pool(name="bst", bufs=2) as bp,
            tc.tile_pool(name="pA", bufs=2, space="PSUM") as pA,
            tc.tile_pool(name="pB", bufs=1, space="PSUM") as pB,
        ):
            # ---- constants ----
            ident = cp.tile([128, 128], dt.float32)
            make_identity(nc, ident[:])
            LT = cp.tile([128, 128], dt.float32)
            nc.gpsimd.memset(LT[:], 0.0)
            nc.gpsimd.affine_select(out=LT[:], in_=LT[:],
                                    compare_op=OP.is_ge, fill=1.0,
                                    base=0, channel_multiplier=1,
                                    pattern=[[-1, 128]])
            iot_i = cp.tile([128, 200], dt.int32)
            nc.gpsimd.iota(iot_i[:], pattern=[[1, 200]], base=0,
                           channel_multiplier=0)
            iotf = cp.tile([128, 200], dt.float32)
            nc.vector.tensor_copy(iotf[:], iot_i[:])
            iotc_i = cp.tile([128, 1], dt.int32)
            nc.gpsimd.iota(iotc_i[:], pattern=[[0, 1]], base=0,
                           channel_multiplier=1)
            iotc = cp.tile([128, 1], dt.float32)
            nc.vector.tensor_copy(iotc[:], iotc_i[:])
            cbase_i = cp.tile([128, 2], dt.int32)
            nc.gpsimd.iota(cbase_i[:], pattern=[[128, 2]], base=-8,
                           channel_multiplier=1)
            cbase = cp.tile([128, 2], dt.float32)
            nc.vector.tensor_copy(cbase[:], cbase_i[:])
            SI = cp.tile([128, 64], dt.float32)
            nc.gpsimd.memset(SI[:], 0.0)
            nc.gpsimd.affine_select(out=SI[:], in_=SI[:],
                                    compare_op=OP.not_equal, fill=1.0,
                                    base=0, channel_multiplier=1,
                                    pattern=[[-1, 64]])
            nc.gpsimd.affine_select(out=SI[:], in_=SI[:],
                                    compare_op=OP.not_equal, fill=1.0,
                                    base=-64, channel_multiplier=1,
                                    pattern=[[-1, 64]])
            onescol = cp.tile([128, 1], dt.float32)
            nc.vector.memset(onescol[:], 1.0)
            epscol = cp.tile([128, 1], dt.float32)
            nc.vector.memset(epscol[:], EPS)
            onesrow = cp.tile([1, 128], dt.float32)
            nc.vector.memset(onesrow[:], 1.0)
            ones11 = onesrow[:, 0:1]
            pos_sb = cp.tile([128, 6, 197], dt.float32)
            nc.gpsimd.dma_start(pos_sb[:], dr["POS"][:])
            pch_sb = cp.tile([128, 6, 196], dt.float32)
            nc.gpsimd.dma_start(pch_sb[:], pch[:])

            # ---- persistent state ----
            xcur = sp.tile([128, 6, 197], dt.float32)
            xn = sp.tile([128, 6, 197], dt.float32)
            macc = sp.tile([128, 6, 197], dt.float32)
            qk = sp.tile([128, 12, 197], dt.float32)
            vtok = sp.tile([128, 2, 768], dt.float32)
            vsz = sp.tile([128, 2, 768], dt.float32)
            Esb = sp.tile([128, 2, 197], dt.float32)
            onrm = sp.tile([128, 6, 197], dt.float32)
            xproc = sp.tile([128, 6, 197], dt.float32)
            xdein = sp.tile([128, 2, 768], dt.float32)
            xszd = sp.tile([128, 2, 768], dt.float32)
            h1 = sp.tile([128, 24, 197], dt.float32)
            h2 = sp.tile([128, 6, 197], dt.float32)
            adhs = sp.tile([ADH, 197], dt.float32)
            lo_sb = sp.tile([LORA, 197], dt.float32)
            met = sp.tile([64, 197], dt.float32)
            mn = sp.tile([64, 197], dt.float32)
            asb = sp.tile([64, 100], dt.float32)
            bsb = sp.tile([64, 100], dt.float32)
            Ssb = sp.tile([100, 100], dt.float32)
            tmp = sp.tile([128, 197], dt.float32)
            scr = sp.tile([128, 16], dt.float32)
            G = sp.tile([128, 2, 197], dt.float32)
            cfull = sp.tile([128, 2], dt.float32)
            rsc = sp.tile([1, M], dt.float32)
            rcol = sp.tile([128, M], dt.float32)
            szc = [sp.tile([128, 2], dt.float32, name=f"szc{m}",
                           tag=f"szc{m}") for m in range(M)]
            szr = [sp.tile([1, 197], dt.float32, name=f"szr{m}",
                           tag=f"szr{m}") for m in range(M)]
            sdei = sp.tile([128, 2], dt.float32)
            ssc = sp.tile([128, 2], dt.float32)
            drow = sp.tile([1, 256], dt.float32)
            stat = sp.tile([1, 197], dt.float32)
            stat2 = sp.tile([1, 197], dt.float32)
            nm8 = sp.tile([100, 8], dt.float32)
            ni8 = sp.tile([100, 8], dt.uint32)
            nmrow = sp.tile([1, 100], dt.float32)
            v8 = sp.tile([1, 8], dt.float32)
            i8u = sp.tile([1, 8], dt.uint32)
            srcf = sp.tile([1, 8], dt.float32)
            oh = sp.tile([100, 8], dt.float32)
            ohT = sp.tile([8, 100], dt.float32)
            kept = sp.tile([100, 1], dt.float32)
            colx = sp.tile([100, 1], dt.float32)
            dsb8 = sp.tile([8, 1], dt.float32)
            nidxf = sp.tile([100, 1], dt.float32)
            ob = sp.tile([1, 700], dt.float32)
            ribs = sp.tile([128, 197], dt.float32)

            def act(o, i_, f=AF.Identity, **kw):
                nc.scalar.activation(o, i_, f, **kw)

            def mm(o, lt, rh, st=True, sp_=True, **kw):
                nc.tensor.matmul(o, lt, rh, start=st, stop=sp_, **kw)

            def ts(o, i0, s1, op0, s2=None, op1=None):
                kw = {}
                if op1 is not None:
                    kw["op1"] = op1
                nc.vector.tensor_scalar(out=o, in0=i0, scalar1=s1, scalar2=s2,
                                        op0=op0, **kw)

            # ================= patch embed =================
            for m in range(M):
                nc.vector.memset(szc[m][:], 1.0)
                nc.vector.memset(szr[m][:], 1.0)
            for cb in range(6):
                acc = pA.tile([128, 512], dt.float32, tag="acc")
                for kb in range(6):
                    w = wp.tile([128, 128], dt.float32, tag="pw")
                    nc.gpsimd.dma_start(w[:], dr["PW"][kb, cb])
                    mm(acc[:, 0:196], w[:], pch_sb[:, kb, :], st=(kb == 0),
                       sp_=(kb == 5))
                nc.vector.tensor_copy(xcur[:, cb, :], pos_sb[:, cb, :])
                nc.vector.tensor_add(out=xcur[:, cb, 1:197], in0=acc[:, 0:196],
                                     in1=pos_sb[:, cb, 1:197])

            # ================= layers =================
            for li in range(n_layers):
                N = 197 - 8 * li
                Na, Nb, Nout = (N + 1) // 2, N // 2, N - 8
                nkb = 2 if N > 128 else 1
                kbsz = [128, N - 128] if nkb == 2 else [N]

                # ---- router scores ----
                rt = bp.tile([128, 6, M], dt.float32, tag="rt")
                nc.gpsimd.dma_start(rt[:], dr["RT"][li])
                rl = pA.tile([1, M], dt.float32, tag="sml")
                for cb in range(6):
                    mm(rl[:], xcur[:, cb, 0:1], rt[:, cb, :], st=(cb == 0),
                       sp_=(cb == 5))
                act(rsc[:], rl[:])
                nc.vector.tensor_reduce(stat[:, 0:1], rsc[:], mybir.AxisListType.X, OP.max)
                ts(stat[:, 1:2], stat[:, 0:1], -1.0, OP.mult)
                act(rsc[:], rsc[:], AF.Exp, bias=stat[:, 1:2])
                nc.vector.tensor_reduce(stat[:, 2:3], rsc[:], mybir.AxisListType.X, OP.add)
                nc.vector.reciprocal(stat[:, 3:4], stat[:, 2:3])
                ts(rsc[:], rsc[:], stat[:, 3:4], OP.mult)
                rcp = pA.tile([128, M], dt.float32, tag="sml")
                mm(rcp[:], onesrow[:], rsc[:])
                nc.vector.tensor_copy(rcol[:], rcp[:])

                # ---- shared LN (no affine; folded into weights) ----
                def layernorm(src, dst, n_):
                    sm = pA.tile([1, 512], dt.float32, tag="sml")
                    s2 = pA.tile([1, 512], dt.float32, tag="sml")
                    for cb in range(6):
                        mm(sm[:, :n_], onescol[:], src[:, cb, :n_],
                           st=(cb == 0), sp_=(cb == 5))
                    for cb in range(6):
                        act(tmp[:, :n_], src[:, cb, :n_], AF.Square)
                        mm(s2[:, :n_], onescol[:], tmp[:, :n_],
                           st=(cb == 0), sp_=(cb == 5))
                    ts(stat[:, :n_], sm[:, :n_], 1.0 / C, OP.mult)
                    ts(stat2[:, :n_], s2[:, :n_], 1.0 / C, OP.mult)
                    nc.vector.tensor_tensor(out=drow[:, :n_],
                                            in0=stat[:, :n_],
                                            in1=stat[:, :n_], op=OP.mult)
                    nc.vector.tensor_sub(stat2[:, :n_], stat2[:, :n_],
                                         drow[:, :n_])
                    act(stat2[:, :n_], stat2[:, :n_], AF.Sqrt, bias=epscol[0:1, :])
                    nc.vector.reciprocal(stat2[:, :n_], stat2[:, :n_])
                    mub = pB.tile([128, 197], dt.float32, tag="bc",
                                  name="mub")
                    mm(mub[:, :n_], onesrow[:], stat[:, :n_])
                    for cb in range(6):
                        nc.vector.tensor_sub(dst[:, cb, :n_], src[:, cb, :n_],
                                             mub[:, :n_])
                    rsb_ = pB.tile([128, 197], dt.float32, tag="bc",
                                   name="rsb_")
                    mm(rsb_[:, :n_], onesrow[:], stat2[:, :n_])
                    for cb in range(6):
                        nc.vector.tensor_tensor(out=dst[:, cb, :n_],
                                                in0=dst[:, cb, :n_],
                                                in1=rsb_[:, :n_], op=OP.mult)

                layernorm(xcur, xn, N)
                if dbg and li == n_layers - 1:
                    nc.gpsimd.dma_start(dxn[:], xn[:])

                # ---- per-model branch ----
                for m in range(M):
                    la_t = bp.tile([128, 6, LORA], dt.float32, tag="la")
                    nc.gpsimd.dma_start(la_t[:], dr["LA"][li, m])
                    lop = pA.tile([LORA, 512], dt.float32, tag="sml")
                    for cb in range(6):
                        mm(lop[:, :N], la_t[:, cb, :], xn[:, cb, :N],
                           st=(cb == 0), sp_=(cb == 5))
                    act(lo_sb[:, :N], lop[:, :N])
                    lbqk = bp.tile([LORA, 1536], dt.float32, tag="lbqk")
                    nc.gpsimd.dma_start(lbqk[:], dr["LBQK"][li, m])
                    bqk = bp.tile([128, 12], dt.float32, tag="bqk")
                    nc.gpsimd.dma_start(bqk[:], dr["BQK"][li, m])
                    for db in range(12):
                        accq = pA.tile([128, 512], dt.float32, tag="acc")
                        for cb in range(6):
                            w = wp.tile([128, 128], dt.float32, tag="wqk")
                            nc.gpsimd.dma_start(w[:], dr["WQK"][li, m, cb, db])
                            mm(accq[:, :N], w[:], xn[:, cb, :N], st=(cb == 0),
                               sp_=False)
                        mm(accq[:, :N], lbqk[:, db * 128:(db + 1) * 128],
                           lo_sb[:, :N], st=False, sp_=True)
                        act(qk[:, db, :N], accq[:, :N], bias=bqk[:, db:db + 1])
                    if dbg and li == n_layers - 1:
                        nc.gpsimd.dma_start(dqk2[m, :, :, 0:N], qk[:, :, :N])
                        nc.gpsimd.dma_start(dlo[m, :, 0:N], lo_sb[:, :N])
                    lbv = bp.tile([LORA, 768], dt.float32, tag="lbv")
                    nc.gpsimd.dma_start(lbv[:], dr["LBV"][li, m])
                    bvr = bp.tile([1, 768], dt.float32, tag="bv")
                    nc.gpsimd.dma_start(bvr[:], dr["BV"][li, m])
                    for kb in range(nkb):
                        nsz = kbsz[kb]
                        t0 = kb * 128
                        for fs in range(2):
                            accv = pA.tile([128, 512], dt.float32, tag="acc")
                            for cb in range(6):
                                w = wp.tile([128, 384], dt.float32, tag="wv")
                                nc.gpsimd.dma_start(w[:],
                                                    dr["WV"][li, m, cb, fs])
                                mm(accv[:nsz, 0:384], xn[:, cb, t0:t0 + nsz],
                                   w[:], st=(cb == 0), sp_=False)
                            mm(accv[:nsz, 0:384], lo_sb[:, t0:t0 + nsz],
                               lbv[:, fs * 384:(fs + 1) * 384], st=False,
                               sp_=False)
                            mm(accv[:nsz, 0:384], onesrow[:, :nsz],
                               bvr[:, fs * 384:(fs + 1) * 384], st=False,
                               sp_=True)
                            act(vtok[:nsz, kb, fs * 384:(fs + 1) * 384],
                                accv[:nsz, 0:384])
                        ts(vsz[:nsz, kb, :], vtok[:nsz, kb, :],
                           szc[m][:nsz, kb:kb + 1], OP.mult)
                    # attention
                    for h in range(12):
                        off = 64 * (h % 2)
                        qb = h // 2
                        kbi = 6 + h // 2
                        st_ = pA.tile([128, 2, 197], dt.float32, tag="st")
                        for kb in range(nkb):
                            nsz = kbsz[kb]
                            mm(st_[:nsz, kb, :N],
                               qk[off:off + 64, kbi, kb * 128:kb * 128 + nsz],
                               qk[off:off + 64, qb, :N])
                            act(Esb[:nsz, kb, :N], st_[:nsz, kb, :N], AF.Exp,
                                scale=SCALE)
                        av = pB.tile([64, 197], dt.float32, tag="av")
                        dn = pA.tile([1, 512], dt.float32, tag="sml")
                        for kb in range(nkb):
                            nsz = kbsz[kb]
                            mm(av[:, :N], vsz[:nsz, kb, h * 64:h * 64 + 64],
                               Esb[:nsz, kb, :N], st=(kb == 0),
                               sp_=(kb == nkb - 1))
                        for kb in range(nkb):
                            nsz = kbsz[kb]
                            mm(dn[:, :N], szc[m][:nsz, kb:kb + 1],
                               Esb[:nsz, kb, :N], st=(kb == 0),
                               sp_=(kb == nkb - 1))
                        act(drow[:, :N], dn[:, :N])
                        dnb = pB.tile([128, 197], dt.float32, tag="bc")
                        mm(dnb[0:64, :N], onesrow[:, 0:64], drow[:, :N])
                        nc.vector.reciprocal(mn[:, :N], dnb[0:64, :N])
                        nc.vector.tensor_tensor(
                            out=onrm[off:off + 64, qb, :N], in0=av[:, :N],
                            in1=mn[:, :N], op=OP.mult)
                    # proj + residual
                    bp_t = bp.tile([128, 6], dt.float32, tag="bp")
                    nc.gpsimd.dma_start(bp_t[:], dr["BP"][li, m])
                    for cb in range(6):
                        accp = pA.tile([128, 512], dt.float32, tag="acc")
                        for ci in range(6):
                            w = wp.tile([128, 128], dt.float32, tag="wp")
                            nc.gpsimd.dma_start(w[:], dr["WP"][li, m, ci, cb])
                            mm(accp[:, :N], w[:], onrm[:, ci, :N],
                               st=(ci == 0), sp_=(ci == 5))
                        nc.vector.scalar_tensor_tensor(
                            out=xproc[:, cb, :N], in0=accp[:, :N],
                            scalar=bp_t[:, cb:cb + 1], in1=xcur[:, cb, :N],
                            op0=OP.add, op1=OP.add)
                    # metric = sum_h k_head via stacked-identity matmul
                    mtp = pB.tile([64, 197], dt.float32, tag="av",
                                  name="mtp")
                    for j in range(6):
                        mm(mtp[:, :N], SI[:], qk[:, 6 + j, :N], st=(j == 0),
                           sp_=(j == 5))
                    act(met[:, :N], mtp[:, :N])
                    act(tmp[0:64, :N], met[:, :N], AF.Square)
                    n2p = pA.tile([1, 512], dt.float32, tag="sml")
                    mm(n2p[:, :N], onescol[0:64, :], tmp[0:64, :N])
                    act(stat[:, :N], n2p[:, :N], AF.Sqrt)
                    nc.vector.reciprocal(stat[:, :N], stat[:, :N])
                    rnb = pB.tile([128, 197], dt.float32, tag="bc")
                    mm(rnb[0:64, :N], onesrow[:, 0:64], stat[:, :N])
                    nc.vector.tensor_tensor(out=mn[:, :N], in0=met[:, :N],
                                            in1=rnb[0:64, :N], op=OP.mult)
                    nc.vector.tensor_copy(asb[:, :Na], mn[:, 0:N:2])
                    nc.vector.tensor_copy(bsb[:, :Nb], mn[:, 1:N:2])
                    sc = pA.tile([128, 512], dt.float32, tag="acc")
                    mm(sc[:Na, :Nb], asb[:, :Na], bsb[:, :Nb])
                    act(Ssb[:Na, :Nb], sc[:Na, :Nb])
                    nc.vector.memset(Ssb[0:1, :Nb], NEG)
                    nc.vector.max_with_indices(nm8[:Na, :], ni8[:Na, :],
                                               Ssb[:Na, :Nb])
                    nc.vector.tensor_copy(nidxf[:Na, :], ni8[:Na, 0:1])
                    nmp = pA.tile([1, 512], dt.float32, tag="sml")
                    mm(nmp[:, :Na], nm8[:Na, 0:1], ident[:Na, :Na])
                    act(nmrow[:, :Na], nmp[:, :Na])
                    nc.vector.max_with_indices(v8[:], i8u[:], nmrow[:, :Na])
                    nc.vector.tensor_copy(srcf[:], i8u[:])
                    scp = pA.tile([8, 1], dt.float32, tag="sml")
                    mm(scp[:], srcf[:], ones11)
                    act(scr[0:8, 0:1], scp[:])
                    sbp = pA.tile([100, 8], dt.float32, tag="sml")
                    mm(sbp[:Na, :], onesrow[:, :Na], srcf[:])
                    ts(oh[:Na, :], sbp[:Na, :], iotc[:Na, :], OP.is_equal)
                    ts(ohT[:, :Na], iotf[0:8, :Na], scr[0:8, 0:1], OP.is_equal)
                    nc.vector.tensor_reduce(kept[:Na, :], oh[:Na, :], mybir.AxisListType.X, OP.add)
                    ts(kept[:Na, :], kept[:Na, :], -1.0, OP.mult, 1.0, OP.add)
                    pop = pA.tile([100, 1], dt.float32, tag="sml")
                    mm(pop[:Na, :], LT[:Na, :Na], kept[:Na, :])
                    act(scr[:Na, 4:5], pop[:Na, :])
                    dsp = pA.tile([8, 1], dt.float32, tag="sml")
                    mm(dsp[:], oh[:Na, :], nidxf[:Na, :])
                    act(dsb8[:], dsp[:])
                    dcp = pA.tile([100, 1], dt.float32, tag="sml")
                    mm(dcp[:Na, :], ohT[:, :Na], dsb8[:])
                    ts(scr[:Na, 5:6], dcp[:Na, :], float(Na - 8), OP.add)
                    nc.vector.tensor_sub(scr[:Na, 6:7], scr[:Na, 4:5],
                                         scr[:Na, 5:6])
                    nc.vector.scalar_tensor_tensor(
                        out=colx[:Na, :], in0=kept[:Na, :],
                        scalar=scr[:Na, 6:7], in1=scr[:Na, 5:6],
                        op0=OP.mult, op1=OP.add)
                    nc.vector.tensor_copy(cfull[0:Na, 0:1], colx[:Na, :])
                    ts(cfull[:, 1:2], iotc[:], float(Na - 8), OP.add)
                    if dbg and li == n_layers - 1:
                        nc.gpsimd.dma_start(dmet[m, :, 0:N], met[:, :N])
                        nc.gpsimd.dma_start(dqk[m, :, 0:N], qk[:, 6, :N])
                        nc.gpsimd.dma_start(dix[m, :, 0:8], srcf[:])
                        nc.gpsimd.dma_start(dix[m, :, 8:16], v8[:])
                        nc.gpsimd.dma_start(dd2[m], dsb8[:])
                        nc.gpsimd.dma_start(dnm[m, :, 0:Na], nmrow[:, 0:Na])
                    ts(G[:Na, 0, :Nout], iotf[:Na, :Nout],
                       cfull[:Na, 0:1], OP.is_equal)
                    ts(G[:Nb, 1, :Nout], iotf[:Nb, :Nout],
                       cfull[:Nb, 1:2], OP.is_equal)
                    # transpose xproc -> deinterleaved token-major
                    # (a-tokens: blk0 partitions 0..Na-1; b: blk1 0..Nb-1)
                    for cb in range(6):
                        tp_ = pA.tile([128, 512], dt.float32, tag="acc")
                        mm(tp_[:Na, 0:128], xproc[:, cb, 0:N:2], ident[:],
                           is_transpose=True)
                        act(xdein[:Na, 0, cb * 128:(cb + 1) * 128],
                            tp_[:Na, 0:128])
                        tq_ = pA.tile([128, 512], dt.float32, tag="acc")
                        mm(tq_[:Nb, 0:128], xproc[:, cb, 1:N:2], ident[:],
                           is_transpose=True)
                        act(xdein[:Nb, 1, cb * 128:(cb + 1) * 128],
                            tq_[:Nb, 0:128])
                    # sizes: deint row -> col; fold rscore into x scale
                    nc.vector.tensor_copy(drow[:, 0:Na], szr[m][:, 0:N:2])
                    nc.vector.tensor_copy(drow[:, 100:100 + Nb],
                                          szr[m][:, 1:N:2])
                    for kb, (nn, f0) in enumerate([(Na, 0), (Nb, 100)]):
                        sdp = pA.tile([128, 1], dt.float32, tag="sml")
                        mm(sdp[:nn, :], drow[:, f0:f0 + nn], ones11)
                        act(sdei[:nn, kb:kb + 1], sdp[:nn, :])
                        ts(ssc[:nn, kb:kb + 1], sdei[:nn, kb:kb + 1],
                           rcol[:nn, m:m + 1], OP.mult)
                        ts(xszd[:nn, kb, :], xdein[:nn, kb, :],
                           ssc[:nn, kb:kb + 1], OP.mult)
                    sop = pA.tile([1, 512], dt.float32, tag="sml")
                    mm(sop[:, :Nout], sdei[:Na, 0:1], G[:Na, 0, :Nout],
                       st=True, sp_=False)
                    mm(sop[:, :Nout], sdei[:Nb, 1:2], G[:Nb, 1, :Nout],
                       st=False, sp_=True)
                    act(szr[m][:, :Nout], sop[:, :Nout])
                    nc.vector.reciprocal(drow[:, :Nout], szr[m][:, :Nout])
                    rib = pB.tile([128, 197], dt.float32, tag="bc")
                    mm(rib[:, :Nout], onesrow[:], drow[:, :Nout])
                    act(ribs[:, :Nout], rib[:, :Nout])
                    for kb in range(2 if Nout > 128 else 1):
                        kk = min(128, Nout - kb * 128)
                        sdp2 = pA.tile([128, 1], dt.float32, tag="sml")
                        mm(sdp2[:kk, :], szr[m][:, kb * 128:kb * 128 + kk],
                           ones11)
                        act(szc[m][:kk, kb:kb + 1], sdp2[:kk, :])
                    for cb in range(6):
                        mg = pA.tile([128, 512], dt.float32, tag="acc")
                        mm(mg[:, :Nout],
                           xszd[:Na, 0, cb * 128:(cb + 1) * 128],
                           G[:Na, 0, :Nout], st=True, sp_=False)
                        mm(mg[:, :Nout],
                           xszd[:Nb, 1, cb * 128:(cb + 1) * 128],
                           G[:Nb, 1, :Nout], st=False, sp_=True)
                        if m == 0:
                            nc.vector.tensor_tensor(out=macc[:, cb, :Nout],
                                                    in0=mg[:, :Nout],
                                                    in1=ribs[:, :Nout],
                                                    op=OP.mult)
                        else:
                            nc.vector.tensor_tensor(out=tmp[:, :Nout],
                                                    in0=mg[:, :Nout],
                                                    in1=ribs[:, :Nout],
                                                    op=OP.mult)
                            nc.vector.tensor_add(out=macc[:, cb, :Nout],
                                                 in0=macc[:, cb, :Nout],
                                                 in1=tmp[:, :Nout])

                # ---- shared MLP + adapter ----
                layernorm(macc, xn, Nout)
                fb1 = bp.tile([128, 24], dt.float32, tag="fb1")
                nc.gpsimd.dma_start(fb1[:], dr["FB1"][li])
                for db in range(24):
                    a1 = pA.tile([128, 512], dt.float32, tag="acc")
                    for cb in range(6):
                        w = wp.tile([128, 128], dt.float32, tag="wf1")
                        nc.gpsimd.dma_start(w[:], dr["FC1"][li, cb, db])
                        mm(a1[:, :Nout], w[:], xn[:, cb, :Nout], st=(cb == 0),
                           sp_=(cb == 5))
                    act(h1[:, db, :Nout], a1[:, :Nout], AF.Gelu,
                        bias=fb1[:, db:db + 1])
                for cb in range(6):
                    a2 = pA.tile([128, 512], dt.float32, tag="acc")
                    for db in range(24):
                        w = wp.tile([128, 128], dt.float32, tag="wf2")
                        nc.gpsimd.dma_start(w[:], dr["FC2"][li, db, cb])
                        mm(a2[:, :Nout], w[:], h1[:, db, :Nout], st=(db == 0),
                           sp_=(db == 23))
                    act(h2[:, cb, :Nout], a2[:, :Nout])
                adw = bp.tile([128, 6, ADH], dt.float32, tag="adw")
                nc.gpsimd.dma_start(adw[:], dr["ADW"][li])
                adb = bp.tile([ADH, 1], dt.float32, tag="adb")
                nc.gpsimd.dma_start(adb[:], dr["ADB"][li])
                adu = bp.tile([ADH, 768], dt.float32, tag="adu")
                nc.gpsimd.dma_start(adu[:], dr["ADU"][li])
                bc_t = bp.tile([128, 6], dt.float32, tag="bc")
                nc.gpsimd.dma_start(bc_t[:], dr["BC"][li])
                ah = pB.tile([64, 197], dt.float32, tag="av")
                for cb in range(6):
                    mm(ah[:, :Nout], adw[:, cb, :], h2[:, cb, :Nout],
                       st=(cb == 0), sp_=(cb == 5))
                act(adhs[:, :Nout], ah[:, :Nout], AF.Relu, bias=adb[:])
                for cb in range(6):
                    au = pA.tile([128, 512], dt.float32, tag="acc")
                    mm(au[:, :Nout], adu[:, cb * 128:(cb + 1) * 128],
                       adhs[:, :Nout])
                    nc.vector.scalar_tensor_tensor(
                        out=tmp[:, :Nout], in0=au[:, :Nout],
                        scalar=bc_t[:, cb:cb + 1], in1=h2[:, cb, :Nout],
                        op0=OP.add, op1=OP.add)
                    nc.vector.tensor_add(out=xcur[:, cb, :Nout],
                                         in0=macc[:, cb, :Nout],
                                         in1=tmp[:, :Nout])

            if dbg:
                nc.gpsimd.dma_start(dxc[:], xcur[:])
                for m in range(M):
                    nc.gpsimd.dma_start(dsz[m], szr[m][:])

            # ================= final LN (cls) + head =================
            hw_sb = cp.tile([128, 6, 700], dt.float32)
            nc.gpsimd.dma_start(hw_sb[:], dr["HW"][:])
            hb_sb = cp.tile([1, 700], dt.float32)
            nc.gpsimd.dma_start(hb_sb[:], dr["HB"][:])
            sm = pA.tile([1, 512], dt.float32, tag="sml")
            for cb in range(6):
                mm(sm[:, 0:1], onescol[:], xcur[:, cb, 0:1], st=(cb == 0),
                   sp_=(cb == 5))
            act(tmp[:, 0:6], xcur[:, :, 0:1], AF.Square)
            s2 = pA.tile([1, 512], dt.float32, tag="sml")
            for cb in range(6):
                mm(s2[:, 0:1], onescol[:], tmp[:, cb:cb + 1], st=(cb == 0),
                   sp_=(cb == 5))
            ts(stat[:, 0:1], sm[:, 0:1], 1.0 / C, OP.mult)
            ts(stat[:, 1:2], s2[:, 0:1], 1.0 / C, OP.mult)
            nc.vector.tensor_tensor(out=stat[:, 2:3], in0=stat[:, 0:1],
                                    in1=stat[:, 0:1], op=OP.mult)
            nc.vector.tensor_sub(stat[:, 1:2], stat[:, 1:2], stat[:, 2:3])
            act(stat[:, 1:2], stat[:, 1:2], AF.Sqrt, bias=epscol[0:1, :])
            nc.vector.reciprocal(stat[:, 1:2], stat[:, 1:2])
            mubp = pB.tile([128, 197], dt.float32, tag="bc", name="mubp")
            mm(mubp[:, 0:1], onesrow[:], stat[:, 0:1])
            act(scr[:, 2:3], mubp[:, 0:1])
            rstp = pB.tile([128, 197], dt.float32, tag="bc", name="rstp")
            mm(rstp[:, 0:1], onesrow[:], stat[:, 1:2])
            act(scr[:, 3:4], rstp[:, 0:1])
            for cb in range(6):
                ts(tmp[:, 8 + cb:9 + cb], xcur[:, cb, 0:1], scr[:, 2:3],
                   OP.subtract, scr[:, 3:4], OP.mult)
            for fs in range(2):
                lg = pA.tile([1, 512], dt.float32, tag="sml")
                for cb in range(6):
                    mm(lg[:, 0:350], tmp[:, 8 + cb:9 + cb],
                       hw_sb[:, cb, fs * 350:(fs + 1) * 350], st=(cb == 0),
                       sp_=(cb == 5))
                nc.vector.tensor_add(out=ob[:, fs * 350:(fs + 1) * 350],
                                     in0=lg[:, 0:350],
                                     in1=hb_sb[:, fs * 350:(fs + 1) * 350])
            nc.gpsimd.dma_start(out[:], ob[:])

    nc.compile()
    return nc


# ----------------------------------------------------------------------------
# Host runner: cached jit via PJRT (axon), weights device-resident.
# ----------------------------------------------------------------------------
_STATE = {}


def _patchify(x):
    xb = x.reshape(B, 3, GRID, P, GRID, P).transpose(0, 1, 3, 5, 2, 4)
    xb = xb.reshape(B, 768, 196)
    return np.ascontiguousarray(
        xb.reshape(B, 6, 128, 196).transpose(0, 2, 1, 3))


def _get_runner(inputs):
    if "runner" in _STATE:
        return _STATE["runner"]
    import jax
    from jax.sharding import Mesh, PartitionSpec, NamedSharding
    from jax.experimental.shard_map import shard_map
    from concourse import bass2jax, mybir as mb

    # Disk-cache walrus NEFF compiles (keyed on BIR bytes) so repeat
    # processes skip the multi-minute compile.
    _orig_compile = bass2jax.compile_bir_kernel

    def _cached_compile(bir_json, tmpdir, neff_name="file.neff"):
        import hashlib, shutil
        key = hashlib.sha256(
            bir_json if isinstance(bir_json, bytes) else bir_json.encode()
        ).hexdigest()[:24]
        cdir = "/tmp/bass_neff_cache"
        os.makedirs(cdir, exist_ok=True)
        cpath = os.path.join(cdir, f"{key}.neff")
        dst = os.path.join(tmpdir, neff_name)
        if os.path.exists(cpath):
            shutil.copy(cpath, dst)
            return dst
        neff = _orig_compile(bir_json, tmpdir, neff_name)
        try:
            shutil.copy(neff, cpath)
        except OSError:
            pass
        return neff

    bass2jax.compile_bir_kernel = _cached_compile

    nc = build_nc()
    bass2jax.install_neuronx_cc_hook()

    partition_name = (nc.partition_id_tensor.name
                      if nc.partition_id_tensor else None)
    in_names, out_names, out_avals, zero_shapes = [], [], [], []
    for alloc in nc.m.functions[0].allocations:
        if not isinstance(alloc, mb.MemoryLocationSet):
            continue
        name = alloc.memorylocations[0].name
        if alloc.kind == "ExternalInput":
            if name != partition_name:
                in_names.append(name)
        elif alloc.kind == "ExternalOutput":
            out_names.append(name)
            shape = tuple(alloc.tensor_shape)
            dtype = mb.dt.np(alloc.dtype)
            out_avals.append(jax.core.ShapedArray(shape, dtype))
            zero_shapes.append((shape, dtype))
    n_params = len(in_names)
    all_in = list(in_names) + list(out_names)
    if partition_name is not None:
        all_in.append(partition_name)
    donate = tuple(range(n_params, n_params + len(out_names)))

    def _body(*args):
        operands = list(args)
        if partition_name is not None:
            operands.append(bass2jax.partition_id_tensor())
        outs = bass2jax._bass_exec_p.bind(
            *operands, out_avals=tuple(out_avals), in_names=tuple(all_in),
            out_names=tuple(out_names), lowering_input_output_aliases=(),
            sim_require_finite=False, sim_require_nnan=False, nc=nc)
        return tuple(outs)

    devices = jax.devices()[:8]
    mesh = Mesh(np.asarray(devices), ("core",))
    in_specs = (PartitionSpec("core"),) * (n_params + len(out_names))
    out_specs = (PartitionSpec("core"),) * len(out_names)
    sharded = jax.jit(
        shard_map(_body, mesh=mesh, in_specs=in_specs, out_specs=out_specs,
                  check_rep=False),
        donate_argnums=donate, keep_unused=True)
    shsh = NamedSharding(mesh, PartitionSpec("core"))

    tables = _prep_tables(inputs)
    dev_tables = {}
    for name in in_names:
        if name == "PCH":
            continue
        arr = tables[name]
        big = np.broadcast_to(arr, (8,) + arr.shape).reshape(
            (8 * arr.shape[0],) + arr.shape[1:])
        dev_tables[name] = jax.device_put(np.ascontiguousarray(big), shsh)
    for v in dev_tables.values():
        v.block_until_ready()

    zeros_np = [np.zeros((8 * s[0],) + tuple(s[1:]), d)
                for s, d in zero_shapes]
    oidx = out_names.index("OUT")

    def run(pch_percore):
        args = []
        pch_dev = jax.device_put(pch_percore.reshape(8 * 128, 6, 196), shsh)
        for name in in_names:
            args.append(pch_dev if name == "PCH" else dev_tables[name])
        zs = [jax.device_put(z, shsh) for z in zeros_np]
        outs = sharded(*args, *zs)
        return np.asarray(outs[oidx]).reshape(8, 700)

    _STATE["runner"] = run
    return run


def kernel(**inputs):
    inputs = {k: np.asarray(v) for k, v in inputs.items()}
    run = _get_runner(inputs)
    pch = _patchify(inputs["x"].astype(np.float32))
    return np.ascontiguousarray(run(pch).astype(np.float32))
